# revision 32
# baseline (speedup 1.0000x reference)
"""ColumnParallelLinear + per-token LoRA (punica add_lora) on 8 NeuronCores.

out = x @ W^T + b + B[idx] @ (A[idx] @ x^T), idx==-1 skips LoRA.

Sharding: tensor-parallel over the output dim (vLLM ColumnParallelLinear):
weight, bias and lora_b are sharded 512-wide per core; lora_a and indices
are replicated. The per-token LoRA shrink (s = A @ x) is sharded over
tokens (256/core) and shared via an on-chip fp8 AllGather; the LoRA expand
is a dense matmul against the routing-masked shrink
(s_masked[t, (l,r)] = (idx[t]==l) * s[t, (l,r)]).

Perf notes (measured on HW):
- The base matmul stays bf16 (pure fp8 fails the 2e-2 gate: measured
  3.9e-2). The shrink/expand run as fp8e4 DoubleRow matmuls (2 K-tiles per
  instruction, 2 elem/cycle), halving their PE occupancy; their
  quantization error only feeds the small LoRA correction (total ~6e-3).
- sm rides at scale s/4 and lora_b at x4 so the expand product lands at
  scale 1: the last 3 groups accumulate their expand directly into the
  base psum group (no separate psum/combine and almost no kernel tail).
- Each DMA queue only runs ~2 transfers at a time, so the three input
  streams ride three queues: shrink inputs on Activation, weights on
  GpSimd, x-groups on SP. A 1-byte guard DMA keeps the SP bulk stream
  from out-competing the shrink inputs that gate the AllGather chain.
- The PE p-state ramps (1.2 -> ~2 GHz) only under continuous execution,
  so the shrink is interleaved with base group 0 and every gap matters.
- The collective has a ~60us one-time bootstrap regardless of issue time;
  expand work trails the base sweep far enough that the AllGather always
  lands first.
"""
import json

import numpy as np
import ml_dtypes

import concourse.bass as bass
import concourse.mybir as mybir
import concourse.tile as tile
from concourse.bass_utils import run_bass_kernel_spmd

T, H, O, L, R = 2048, 4096, 4096, 32, 16
N_CORES = 8
O_SH = O // N_CORES          # 512  output cols per core
T_LOC = T // N_CORES         # 256  tokens whose LoRA-shrink this core computes
KB = H // 128                # 32   contraction blocks
LR = L * R                   # 512  stacked (lora, rank) rows
G = 8                        # base-matmul token groups
TG = T // G                  # 256  tokens per group
N_FUSED = 2                  # trailing groups whose expand accumulates in-psum
BF16 = mybir.dt.bfloat16
F32 = mybir.dt.float32
F8 = mybir.dt.float8e4
DR = mybir.MatmulPerfMode.DoubleRow
A_SCALE = 16.0               # lora_a is pre-scaled x16 into fp8
B_SCALE = 4.0                # lora_b is pre-scaled x4 into fp8
SM_SCALE = 1.0 / (4.0 * A_SCALE)   # psum_s (16*s) -> sm (s/4); s/4 * 4b = s*b


def _split_waits(raw: bytes) -> bytes:
    """This walrus build rejects instructions carrying more than one sync
    wait ("Too many sync wait commands"), but Tile attaches one wait per
    producing proc. Hoist all but one wait of each instruction onto
    single-wait NoOps inserted just before it on the same engine — the
    engine executes its stream in order, so the gating is identical."""
    m = json.loads(raw)
    ctr = 0
    for f in m["functions"]:
        for b in f["blocks"]:
            out = []
            for inst in b["instructions"]:
                si = inst.get("sync_info")
                waits = si.get("on_wait") if si else None
                if waits and len(waits) > 1:
                    for w in waits[:-1]:
                        ctr += 1
                        out.append({
                            "debug": inst.get("debug", 0),
                            "engine": inst["engine"],
                            "ins": [],
                            "name": f"I-wsplit-{ctr}",
                            "opcode": "NoOp",
                            "outs": [],
                            "sync_info": {"on_update": [], "on_wait": [w]},
                        })
                    si["on_wait"] = [waits[-1]]
                out.append(inst)
            b["instructions"] = out
    return json.dumps(m).encode()


class _WaitSplitBass(bass.Bass):
    def to_json_bytes(self) -> bytes:
        return _split_waits(super().to_json_bytes())


def _build() -> bass.Bass:
    nc = _WaitSplitBass()
    # all streamed inputs are PE-tile-major: [128 h-partitions, ..., free]
    xG = nc.dram_tensor("xG", [128, G, KB, TG], BF16, kind="ExternalInput")
    xl_r = nc.dram_tensor("xl_r", [128, KB, T_LOC], F8, kind="ExternalInput")
    wTr = nc.dram_tensor("wTr", [128, KB, O_SH], BF16, kind="ExternalInput")
    aTr = nc.dram_tensor("aTr", [128, KB, LR], F8, kind="ExternalInput")
    bTr = nc.dram_tensor("bTr", [128, 4, O_SH], F8, kind="ExternalInput")
    bias_row = nc.dram_tensor("bias_row", [1, O_SH], BF16, kind="ExternalInput")
    idx_bc_d = nc.dram_tensor("idx_bc", [128, T_LOC], F32, kind="ExternalInput")
    lrow_d = nc.dram_tensor("lrow", [128, 4], F32, kind="ExternalInput")
    out = nc.dram_tensor("out", [T, O_SH], BF16, kind="ExternalOutput")

    with tile.TileContext(nc) as tc:
        with (
            tc.tile_pool(name="res", bufs=1) as res,          # long-lived SBUF
            tc.tile_pool(name="stream", bufs=4) as stream,    # streamed SBUF
            tc.tile_pool(name="ps", bufs=2, space="PSUM") as ps,
            tc.tile_pool(name="dram", bufs=1, space="DRAM") as dram,
        ):
            # ------------- tiny mask inputs first, then the shrink inputs,
            # all on the Activation queue.  The AllGather completion is
            # bootstrap-floored at ~95-110us, so the shrink stream needs no
            # early bandwidth: the base-matmul x/w streams get priority ------
            bias_r = res.tile([1, O_SH], BF16, name="bias_r")
            nc.scalar.dma_start(bias_r[:], bias_row[:])
            xl_all = res.tile([128, KB, T_LOC], F8, name="xl_all")
            at_all = res.tile([128, KB, LR], F8, name="at_all")
            for lo, hi in ((0, 8), (8, 16), (16, 24), (24, 32)):
                nc.scalar.dma_start(xl_all[:, lo:hi, :], xl_r[:, lo:hi, :])
            for lo, hi in ((0, 8), (8, 16), (16, 24), (24, 32)):
                nc.scalar.dma_start(at_all[:, lo:hi, :], aTr[:, lo:hi, :])
            idx_bc = res.tile([128, T_LOC], F32, name="idx_bc_t")
            nc.scalar.dma_start(idx_bc[:], idx_bc_d[:])
            lrow = res.tile([128, 4], F32, name="lrow_t")
            nc.scalar.dma_start(lrow[:], lrow_d[:])

            # ------------- weights + lora_b on the GpSimd queue -------------
            wt_all = res.tile([128, KB, O_SH], BF16, name="wt_all")
            for c in range(4):
                nc.gpsimd.dma_start(wt_all[:, 8 * c:8 * (c + 1), :],
                                    wTr[:, 8 * c:8 * (c + 1), :])
            bt_all = res.tile([128, 4, O_SH], F8, name="bt_all")
            nc.gpsimd.dma_start(bt_all[:], bTr[:])

            # ------------- x groups on the SP queue -------------------------
            xs0 = stream.tile([128, KB, TG], BF16, name="xs", tag="xs", bufs=3)
            nc.sync.dma_start(xs0[:, 0:16, :], xG[:, 0:1, 0:16, :])
            nc.sync.dma_start(xs0[:, 16:32, :], xG[:, 0:1, 16:32, :])

            ones_t = res.tile([1, O_SH], BF16, name="ones_t")
            nc.vector.memset(ones_t[:], 1.0)

            # base accumulations of non-fused groups land in SBUF (with bias)
            # as each group finishes; their expand is combined during store.
            base_sb = res.tile([128, 2 * (G - N_FUSED) * O_SH], F32,
                               name="base_sb")

            ps_s = [ps.tile([128, T_LOC], F32, name=f"ps_s{m}", tag=tg)
                    for m, tg in enumerate(["psd0", "psd1", "psd0", "psd1"])]

            # Preamble: bias broadcast (K=1 ones-matmul, also warms the PE),
            # then base groups 0-1 on the early bandwidth; the fp8 DoubleRow
            # shrink (each matmul contracts the kb pair (2k2, 2k2+1)) runs
            # after group 1, warm and with its trickled inputs surely landed.
            bias_ps = ps.tile([128, O_SH], F32, name="bias_ps", tag="pso0")
            nc.tensor.matmul(bias_ps[:], ones_t[:, 0:128], bias_r[:],
                             start=True, stop=True)
            bias_bc = res.tile([128, O_SH], F32, name="bias_bc")
            nc.vector.tensor_copy(bias_bc[:], bias_ps[:])

            ps_o0 = [ps.tile([128, O_SH], F32, name=f"ps_o0_{t}", tag=f"pso{t}")
                     for t in range(2)]

            def base0_chunk(lo, hi):
                for kb in range(lo, hi):
                    for tt in range(2):
                        nc.tensor.matmul(
                            ps_o0[tt][:],
                            xs0[:, kb, tt * 128:(tt + 1) * 128],
                            wt_all[:, kb, :],
                            start=(kb == 0),
                            stop=(kb == KB - 1),
                        )

            def shrink_chunk(c):
                for k2 in range(4 * c, 4 * (c + 1)):
                    for m in range(4):
                        nc.tensor.matmul(
                            ps_s[m][:],
                            at_all[:, 2 * k2:2 * k2 + 2, m * 128:(m + 1) * 128],
                            xl_all[:, 2 * k2:2 * k2 + 2, :],
                            start=(k2 == 0),
                            stop=(k2 == 15),
                            perf_mode=DR,
                        )

            base0_chunk(0, 32)
            for tt in range(2):
                nc.vector.tensor_tensor(
                    base_sb[:, tt * O_SH:(tt + 1) * O_SH],
                    ps_o0[tt][:],
                    bias_bc[:],
                    op=mybir.AluOpType.add,
                )

            def mask_and_allgather():
                # routing mask, rescale + fp8 downcast: sm = (idx==l(p)) * s/4
                sm_f32 = res.tile([128, 4 * T_LOC], F32, name="sm_f32")
                for m in range(4):
                    nc.vector.scalar_tensor_tensor(
                        sm_f32[:, m * T_LOC:(m + 1) * T_LOC],
                        idx_bc[:],
                        lrow[:, m:m + 1],
                        ps_s[m][:],
                        op0=mybir.AluOpType.is_equal,
                        op1=mybir.AluOpType.mult,
                    )
                sm = res.tile([128, 4 * T_LOC], F8, name="sm")
                nc.vector.tensor_scalar_mul(sm[:], sm_f32[:], SM_SCALE)
                cc_in = dram.tile([LR, T_LOC], F8, name="cc_in")
                nc.gpsimd.dma_start(
                    cc_in[:].rearrange("(m p) t -> p m t", p=128),
                    sm[:].rearrange("p (m t) -> p m t", t=T_LOC),
                )
                cc_out = dram.tile([N_CORES, LR, T_LOC], F8, name="cc_out",
                                   addr_space="Shared")
                nc.gpsimd.collective_compute(
                    "AllGather",
                    mybir.AluOpType.bypass,
                    replica_groups=[list(range(N_CORES))],
                    ins=[cc_in.opt()],
                    outs=[cc_out.opt()],
                )
                return cc_out

            cc_out = [None]

            def st_load(g):
                st = stream.tile([128, 4, TG], F8, name="st", tag="st", bufs=4)
                for blk in range(4):
                    # scalar queue: idle by now, and unlike the gpsimd queue
                    # it is not blocked behind the collective
                    nc.scalar.dma_start(
                        st[:, blk:blk + 1, :],
                        cc_out[0][g, blk * 128:(blk + 1) * 128, :],
                    )
                return st

            def expand_mms(ps_into, st, start, stop):
                for tt in range(2):
                    for d2 in range(2):
                        nc.tensor.matmul(
                            ps_into[tt][:],
                            st[:, 2 * d2:2 * d2 + 2, tt * 128:(tt + 1) * 128],
                            bt_all[:, 2 * d2:2 * d2 + 2, :],
                            start=start and (d2 == 0),
                            stop=stop and (d2 == 1),
                            perf_mode=DR,
                        )

            def base_group(g, fused_st=None):
                ps_o = [ps.tile([128, O_SH], F32, name=f"ps_o{g}_{t}",
                                tag=f"pso{t}") for t in range(2)]
                xs = stream.tile([128, KB, TG], BF16, name="xs", tag="xs", bufs=3)
                if g <= 2:
                    # halves: the first 16 kb land ~5us earlier in the
                    # bandwidth-crunched early window
                    nc.sync.dma_start(xs[:, 0:16, :], xG[:, g:g + 1, 0:16, :])
                    nc.sync.dma_start(xs[:, 16:32, :], xG[:, g:g + 1, 16:32, :])
                else:
                    nc.sync.dma_start(xs[:], xG[:, g:g + 1, :, :])
                for kb in range(KB):
                    for tt in range(2):
                        nc.tensor.matmul(
                            ps_o[tt][:],
                            xs[:, kb, tt * 128:(tt + 1) * 128],
                            wt_all[:, kb, :],
                            start=(kb == 0),
                            stop=(fused_st is None) and (kb == KB - 1),
                        )
                if fused_st is not None:
                    # expand accumulates straight into the base psum (sm and
                    # bt scales multiply to exactly 1), then bias + store
                    expand_mms(ps_o, fused_st, start=False, stop=True)
                    ot = stream.tile([128, 2 * O_SH], BF16, name="ot",
                                     tag="ot", bufs=2)
                    for tt in range(2):
                        nc.vector.tensor_tensor(
                            ot[:, tt * O_SH:(tt + 1) * O_SH],
                            ps_o[tt][:],
                            bias_bc[:],
                            op=mybir.AluOpType.add,
                        )
                    for tt in range(2):
                        nc.sync.dma_start(
                            out[g * TG + tt * 128:g * TG + (tt + 1) * 128, :],
                            ot[:, tt * O_SH:(tt + 1) * O_SH])
                else:
                    for tt in range(2):
                        nc.vector.tensor_tensor(
                            base_sb[:, (2 * g + tt) * O_SH:
                                    (2 * g + tt + 1) * O_SH],
                            ps_o[tt][:],
                            bias_bc[:],
                            op=mybir.AluOpType.add,
                        )

            def tail(g, st):
                ps_d = [
                    ps.tile([128, O_SH], F32, name=f"ps_d{g}_{t}", tag=f"psd{t}")
                    for t in range(2)
                ]
                expand_mms(ps_d, st, start=True, stop=True)
                ot = stream.tile([128, 2 * O_SH], BF16, name="ot", tag="ot", bufs=2)
                for tt in range(2):
                    nc.vector.scalar_tensor_tensor(
                        ot[:, tt * O_SH:(tt + 1) * O_SH],
                        ps_d[tt][:],
                        1.0,
                        base_sb[:, (2 * g + tt) * O_SH:(2 * g + tt + 1) * O_SH],
                        op0=mybir.AluOpType.mult,
                        op1=mybir.AluOpType.add,
                    )
                dst = out[g * TG:(g + 1) * TG, :].rearrange(
                    "(tt p) o -> p tt o", p=128
                )
                nc.sync.dma_start(dst, ot[:].rearrange("p (tt o) -> p tt o", o=O_SH))

            # flow: groups 0-5 plain, 6-7 fused.  Shrink chunks bridge the
            # windows where the next x chunk is still in flight; separate
            # expand tails slot in from group 5 on, by when the AllGather
            # has always landed.
            base_group(1)
            for c in range(4):
                shrink_chunk(c)
            cc_out[0] = mask_and_allgather()
            base_group(2)
            base_group(3)
            base_group(4)
            base_group(5)
            # all AllGather-dependent work sits after base sweep 6 (~135us):
            # the collective's completion fluctuates between ~95 and ~128us
            # run-to-run, and a stalled tail would drop the PE p-state
            sts = [st_load(g) for g in (6, 0, 1)]
            base_group(6, fused_st=sts[0])
            tail(0, sts[1])
            tail(1, sts[2])
            sts = [st_load(g) for g in (7, 2, 3)]
            base_group(7, fused_st=sts[0])
            tail(2, sts[1])
            tail(3, sts[2])
            sts = [st_load(g) for g in (4, 5)]
            tail(4, sts[0])
            tail(5, sts[1])
    return nc


_NC_CACHE = None


def build_in_maps(x, weight, bias, lora_a, lora_b, indices):
    bf = ml_dtypes.bfloat16
    f8 = mybir.dt.np(F8)

    # [128 h-partitions, group, kb, token] PE-tile-major layout
    xG = np.ascontiguousarray(
        x.astype(bf).reshape(G, TG, KB, 128).transpose(3, 0, 2, 1))
    aTr = np.ascontiguousarray(
        (lora_a * A_SCALE).astype(f8).reshape(LR, H).T
        .reshape(KB, 128, LR).transpose(1, 0, 2))                   # (128,KB,LR)
    idx_f = indices.astype(np.float32)                              # (T,)
    lrow = np.broadcast_to(
        (np.arange(128)[:, None] // 16).astype(np.float32), (128, 4)
    ).copy()
    lrow = lrow + (np.arange(4)[None, :] * 8).astype(np.float32)    # (128, 4)

    in_maps = []
    for c in range(N_CORES):
        wTc = np.ascontiguousarray(
            weight[c * O_SH:(c + 1) * O_SH, :].astype(bf).T
            .reshape(KB, 128, O_SH).transpose(1, 0, 2))             # (128,KB,O_SH)
        bTc = np.ascontiguousarray(
            (lora_b[:, c * O_SH:(c + 1) * O_SH, :] * B_SCALE).astype(f8)
            .transpose(0, 2, 1).reshape(LR, O_SH)                   # ((l,r), o)
            .reshape(4, 128, O_SH).transpose(1, 0, 2))              # (128,4,O_SH)
        bias_c = np.ascontiguousarray(
            bias[c * O_SH:(c + 1) * O_SH].astype(bf))[None, :]
        idx_bc = np.broadcast_to(
            idx_f[c * T_LOC:(c + 1) * T_LOC][None, :], (128, T_LOC)
        ).copy()
        xl_c = np.ascontiguousarray(
            x[c * T_LOC:(c + 1) * T_LOC, :].astype(f8).T
            .reshape(KB, 128, T_LOC).transpose(1, 0, 2))            # (128,KB,T_LOC)
        in_maps.append({
            "xG": xG, "xl_r": xl_c, "wTr": wTc, "aTr": aTr, "bTr": bTc,
            "bias_row": bias_c, "idx_bc": idx_bc, "lrow": lrow,
        })
    return in_maps


def kernel(x, weight, bias, lora_a, lora_b, indices):
    global _NC_CACHE
    in_maps = build_in_maps(x, weight, bias, lora_a, lora_b, indices)
    if _NC_CACHE is None:
        _NC_CACHE = _build()
    r = run_bass_kernel_spmd(_NC_CACHE, in_maps, core_ids=list(range(N_CORES)))
    return np.concatenate(
        [r.results[c]["out"].astype(np.float32) for c in range(N_CORES)], axis=1)


# revision 33
# speedup vs baseline: 1.0154x; 1.0154x over previous
"""ColumnParallelLinear + per-token LoRA (punica add_lora) on 8 NeuronCores.

out = x @ W^T + b + B[idx] @ (A[idx] @ x^T), idx==-1 skips LoRA.

Sharding: tensor-parallel over the output dim (vLLM ColumnParallelLinear):
weight, bias and lora_b are sharded 512-wide per core; lora_a and indices
are replicated. The per-token LoRA shrink (s = A @ x) is sharded over
tokens (256/core) and shared via an on-chip fp8 AllGather; the LoRA expand
is a dense matmul against the routing-masked shrink
(s_masked[t, (l,r)] = (idx[t]==l) * s[t, (l,r)]).

Perf notes (measured on HW):
- The base matmul stays bf16 (pure fp8 fails the 2e-2 gate: measured
  3.9e-2). The shrink/expand run as fp8e4 DoubleRow matmuls (2 K-tiles per
  instruction, 2 elem/cycle), halving their PE occupancy; their
  quantization error only feeds the small LoRA correction (total ~6e-3).
- sm rides at scale s/4 and lora_b at x4 so the expand product lands at
  scale 1: the last 3 groups accumulate their expand directly into the
  base psum group (no separate psum/combine and almost no kernel tail).
- Each DMA queue only runs ~2 transfers at a time, so the three input
  streams ride three queues: shrink inputs on Activation, weights on
  GpSimd, x-groups on SP. A 1-byte guard DMA keeps the SP bulk stream
  from out-competing the shrink inputs that gate the AllGather chain.
- The PE p-state ramps (1.2 -> ~2 GHz) only under continuous execution,
  so the shrink is interleaved with base group 0 and every gap matters.
- The collective has a ~60us one-time bootstrap regardless of issue time;
  expand work trails the base sweep far enough that the AllGather always
  lands first.
"""
import json

import numpy as np
import ml_dtypes

import concourse.bass as bass
import concourse.mybir as mybir
import concourse.tile as tile
from concourse.bass_utils import run_bass_kernel_spmd

T, H, O, L, R = 2048, 4096, 4096, 32, 16
N_CORES = 8
O_SH = O // N_CORES          # 512  output cols per core
T_LOC = T // N_CORES         # 256  tokens whose LoRA-shrink this core computes
KB = H // 128                # 32   contraction blocks
LR = L * R                   # 512  stacked (lora, rank) rows
G = 8                        # base-matmul token groups
TG = T // G                  # 256  tokens per group
N_FUSED = 2                  # trailing groups whose expand accumulates in-psum
BF16 = mybir.dt.bfloat16
F32 = mybir.dt.float32
F8 = mybir.dt.float8e4
DR = mybir.MatmulPerfMode.DoubleRow
A_SCALE = 16.0               # lora_a is pre-scaled x16 into fp8
B_SCALE = 4.0                # lora_b is pre-scaled x4 into fp8
SM_SCALE = 1.0 / (4.0 * A_SCALE)   # psum_s (16*s) -> sm (s/4); s/4 * 4b = s*b


def _split_waits(raw: bytes) -> bytes:
    """This walrus build rejects instructions carrying more than one sync
    wait ("Too many sync wait commands"), but Tile attaches one wait per
    producing proc. Hoist all but one wait of each instruction onto
    single-wait NoOps inserted just before it on the same engine — the
    engine executes its stream in order, so the gating is identical."""
    m = json.loads(raw)
    ctr = 0
    for f in m["functions"]:
        for b in f["blocks"]:
            out = []
            for inst in b["instructions"]:
                si = inst.get("sync_info")
                waits = si.get("on_wait") if si else None
                if waits and len(waits) > 1:
                    for w in waits[:-1]:
                        ctr += 1
                        out.append({
                            "debug": inst.get("debug", 0),
                            "engine": inst["engine"],
                            "ins": [],
                            "name": f"I-wsplit-{ctr}",
                            "opcode": "NoOp",
                            "outs": [],
                            "sync_info": {"on_update": [], "on_wait": [w]},
                        })
                    si["on_wait"] = [waits[-1]]
                out.append(inst)
            b["instructions"] = out
    return json.dumps(m).encode()


class _WaitSplitBass(bass.Bass):
    def to_json_bytes(self) -> bytes:
        return _split_waits(super().to_json_bytes())


def _build() -> bass.Bass:
    nc = _WaitSplitBass()
    # all streamed inputs are PE-tile-major: [128 h-partitions, ..., free]
    xG = nc.dram_tensor("xG", [128, G, KB, TG], BF16, kind="ExternalInput")
    xl_r = nc.dram_tensor("xl_r", [128, KB, T_LOC], F8, kind="ExternalInput")
    wTr = nc.dram_tensor("wTr", [128, KB, O_SH], BF16, kind="ExternalInput")
    aTr = nc.dram_tensor("aTr", [128, KB, LR], F8, kind="ExternalInput")
    bTr = nc.dram_tensor("bTr", [128, 4, O_SH], F8, kind="ExternalInput")
    bias_row = nc.dram_tensor("bias_row", [1, O_SH], BF16, kind="ExternalInput")
    idx_bc_d = nc.dram_tensor("idx_bc", [128, T_LOC], F32, kind="ExternalInput")
    lrow_d = nc.dram_tensor("lrow", [128, 4], F32, kind="ExternalInput")
    out = nc.dram_tensor("out", [T, O_SH], BF16, kind="ExternalOutput")

    with tile.TileContext(nc) as tc:
        with (
            tc.tile_pool(name="res", bufs=1) as res,          # long-lived SBUF
            tc.tile_pool(name="stream", bufs=4) as stream,    # streamed SBUF
            tc.tile_pool(name="ps", bufs=2, space="PSUM") as ps,
            tc.tile_pool(name="dram", bufs=1, space="DRAM") as dram,
        ):
            # ------------- tiny mask inputs first, then the shrink inputs,
            # all on the Activation queue.  The AllGather completion is
            # bootstrap-floored at ~95-110us, so the shrink stream needs no
            # early bandwidth: the base-matmul x/w streams get priority ------
            bias_r = res.tile([1, O_SH], BF16, name="bias_r")
            nc.scalar.dma_start(bias_r[:], bias_row[:])
            xl_all = res.tile([128, KB, T_LOC], F8, name="xl_all")
            at_all = res.tile([128, KB, LR], F8, name="at_all")
            for lo, hi in ((0, 8), (8, 16), (16, 24), (24, 32)):
                nc.scalar.dma_start(xl_all[:, lo:hi, :], xl_r[:, lo:hi, :])
            for lo, hi in ((0, 8), (8, 16), (16, 24), (24, 32)):
                nc.scalar.dma_start(at_all[:, lo:hi, :], aTr[:, lo:hi, :])
            idx_bc = res.tile([128, T_LOC], F32, name="idx_bc_t")
            nc.scalar.dma_start(idx_bc[:], idx_bc_d[:])
            lrow = res.tile([128, 4], F32, name="lrow_t")
            nc.scalar.dma_start(lrow[:], lrow_d[:])

            # ------------- weights + lora_b on the GpSimd queue -------------
            wt_all = res.tile([128, KB, O_SH], BF16, name="wt_all")
            for c in range(4):
                nc.gpsimd.dma_start(wt_all[:, 8 * c:8 * (c + 1), :],
                                    wTr[:, 8 * c:8 * (c + 1), :])
            bt_all = res.tile([128, 4, O_SH], F8, name="bt_all")
            nc.gpsimd.dma_start(bt_all[:], bTr[:])

            # ------------- x groups on the SP queue -------------------------
            xs0 = stream.tile([128, KB, TG], BF16, name="xs", tag="xs", bufs=3)
            nc.sync.dma_start(xs0[:, 0:16, :], xG[:, 0:1, 0:16, :])
            nc.sync.dma_start(xs0[:, 16:32, :], xG[:, 0:1, 16:32, :])

            ones_t = res.tile([1, O_SH], BF16, name="ones_t")
            nc.vector.memset(ones_t[:], 1.0)

            # base accumulations of non-fused groups land in SBUF (with bias)
            # as each group finishes; their expand is combined during store.
            base_sb = res.tile([128, 2 * (G - N_FUSED) * O_SH], F32,
                               name="base_sb")

            ps_s = [ps.tile([128, T_LOC], F32, name=f"ps_s{m}", tag=tg)
                    for m, tg in enumerate(["psd0", "psd1", "psd0", "psd1"])]

            # Preamble: bias broadcast (K=1 ones-matmul, also warms the PE),
            # then base groups 0-1 on the early bandwidth; the fp8 DoubleRow
            # shrink (each matmul contracts the kb pair (2k2, 2k2+1)) runs
            # after group 1, warm and with its trickled inputs surely landed.
            bias_ps = ps.tile([128, O_SH], F32, name="bias_ps", tag="pso0")
            nc.tensor.matmul(bias_ps[:], ones_t[:, 0:128], bias_r[:],
                             start=True, stop=True)
            bias_bc = res.tile([128, O_SH], F32, name="bias_bc")
            nc.vector.tensor_copy(bias_bc[:], bias_ps[:])

            ps_o0 = [ps.tile([128, O_SH], F32, name=f"ps_o0_{t}", tag=f"pso{t}")
                     for t in range(2)]

            def base0_chunk(lo, hi):
                for kb in range(lo, hi):
                    for tt in range(2):
                        nc.tensor.matmul(
                            ps_o0[tt][:],
                            xs0[:, kb, tt * 128:(tt + 1) * 128],
                            wt_all[:, kb, :],
                            start=(kb == 0),
                            stop=(kb == KB - 1),
                        )

            def shrink_chunk(c):
                for k2 in range(4 * c, 4 * (c + 1)):
                    for m in range(4):
                        nc.tensor.matmul(
                            ps_s[m][:],
                            at_all[:, 2 * k2:2 * k2 + 2, m * 128:(m + 1) * 128],
                            xl_all[:, 2 * k2:2 * k2 + 2, :],
                            start=(k2 == 0),
                            stop=(k2 == 15),
                            perf_mode=DR,
                        )

            base0_chunk(0, 32)
            for tt in range(2):
                nc.vector.tensor_tensor(
                    base_sb[:, tt * O_SH:(tt + 1) * O_SH],
                    ps_o0[tt][:],
                    bias_bc[:],
                    op=mybir.AluOpType.add,
                )

            def mask_and_allgather():
                # routing mask, rescale + fp8 downcast: sm = (idx==l(p)) * s/4
                sm_f32 = res.tile([128, 4 * T_LOC], F32, name="sm_f32")
                for m in range(4):
                    nc.vector.scalar_tensor_tensor(
                        sm_f32[:, m * T_LOC:(m + 1) * T_LOC],
                        idx_bc[:],
                        lrow[:, m:m + 1],
                        ps_s[m][:],
                        op0=mybir.AluOpType.is_equal,
                        op1=mybir.AluOpType.mult,
                    )
                sm = res.tile([128, 4 * T_LOC], F8, name="sm")
                nc.vector.tensor_scalar_mul(sm[:], sm_f32[:], SM_SCALE)
                cc_in = dram.tile([LR, T_LOC], F8, name="cc_in")
                nc.gpsimd.dma_start(
                    cc_in[:].rearrange("(m p) t -> p m t", p=128),
                    sm[:].rearrange("p (m t) -> p m t", t=T_LOC),
                )
                cc_out = dram.tile([N_CORES, LR, T_LOC], F8, name="cc_out",
                                   addr_space="Shared")
                nc.gpsimd.collective_compute(
                    "AllGather",
                    mybir.AluOpType.bypass,
                    replica_groups=[list(range(N_CORES))],
                    ins=[cc_in.opt()],
                    outs=[cc_out.opt()],
                )
                return cc_out

            cc_out = [None]

            def st_load(g):
                st = stream.tile([128, 4, TG], F8, name="st", tag="st", bufs=4)
                for blk in range(4):
                    # scalar queue: idle by now, and unlike the gpsimd queue
                    # it is not blocked behind the collective
                    nc.scalar.dma_start(
                        st[:, blk:blk + 1, :],
                        cc_out[0][g, blk * 128:(blk + 1) * 128, :],
                    )
                return st

            def expand_mms(ps_into, st, start, stop):
                for tt in range(2):
                    for d2 in range(2):
                        nc.tensor.matmul(
                            ps_into[tt][:],
                            st[:, 2 * d2:2 * d2 + 2, tt * 128:(tt + 1) * 128],
                            bt_all[:, 2 * d2:2 * d2 + 2, :],
                            start=start and (d2 == 0),
                            stop=stop and (d2 == 1),
                            perf_mode=DR,
                        )

            def base_group(g, fused_st=None):
                ps_o = [ps.tile([128, O_SH], F32, name=f"ps_o{g}_{t}",
                                tag=f"pso{t}") for t in range(2)]
                xs = stream.tile([128, KB, TG], BF16, name="xs", tag="xs", bufs=3)
                if g <= 2:
                    # halves: the first 16 kb land ~5us earlier in the
                    # bandwidth-crunched early window
                    nc.sync.dma_start(xs[:, 0:16, :], xG[:, g:g + 1, 0:16, :])
                    nc.sync.dma_start(xs[:, 16:32, :], xG[:, g:g + 1, 16:32, :])
                else:
                    nc.sync.dma_start(xs[:], xG[:, g:g + 1, :, :])
                for kb in range(KB):
                    for tt in range(2):
                        nc.tensor.matmul(
                            ps_o[tt][:],
                            xs[:, kb, tt * 128:(tt + 1) * 128],
                            wt_all[:, kb, :],
                            start=(kb == 0),
                            stop=(fused_st is None) and (kb == KB - 1),
                        )
                if fused_st is not None:
                    # expand accumulates straight into the base psum (sm and
                    # bt scales multiply to exactly 1), then bias + store
                    expand_mms(ps_o, fused_st, start=False, stop=True)
                    ot = stream.tile([128, 2 * O_SH], BF16, name="ot",
                                     tag="ot", bufs=4)
                    for tt in range(2):
                        nc.vector.tensor_tensor(
                            ot[:, tt * O_SH:(tt + 1) * O_SH],
                            ps_o[tt][:],
                            bias_bc[:],
                            op=mybir.AluOpType.add,
                        )
                    for tt in range(2):
                        nc.sync.dma_start(
                            out[g * TG + tt * 128:g * TG + (tt + 1) * 128, :],
                            ot[:, tt * O_SH:(tt + 1) * O_SH])
                else:
                    for tt in range(2):
                        nc.vector.tensor_tensor(
                            base_sb[:, (2 * g + tt) * O_SH:
                                    (2 * g + tt + 1) * O_SH],
                            ps_o[tt][:],
                            bias_bc[:],
                            op=mybir.AluOpType.add,
                        )

            def tail(g, st):
                ps_d = [
                    ps.tile([128, O_SH], F32, name=f"ps_d{g}_{t}", tag=f"psd{t}")
                    for t in range(2)
                ]
                expand_mms(ps_d, st, start=True, stop=True)
                ot = stream.tile([128, 2 * O_SH], BF16, name="ot", tag="ot", bufs=4)
                for tt in range(2):
                    nc.vector.scalar_tensor_tensor(
                        ot[:, tt * O_SH:(tt + 1) * O_SH],
                        ps_d[tt][:],
                        1.0,
                        base_sb[:, (2 * g + tt) * O_SH:(2 * g + tt + 1) * O_SH],
                        op0=mybir.AluOpType.mult,
                        op1=mybir.AluOpType.add,
                    )
                dst = out[g * TG:(g + 1) * TG, :].rearrange(
                    "(tt p) o -> p tt o", p=128
                )
                nc.sync.dma_start(dst, ot[:].rearrange("p (tt o) -> p tt o", o=O_SH))

            # flow: groups 0-5 plain, 6-7 fused.  Shrink chunks bridge the
            # windows where the next x chunk is still in flight; separate
            # expand tails slot in from group 5 on, by when the AllGather
            # has always landed.
            base_group(1)
            for c in range(4):
                shrink_chunk(c)
            cc_out[0] = mask_and_allgather()
            base_group(2)
            base_group(3)
            base_group(4)
            base_group(5)
            # all AllGather-dependent work sits after base sweep 6 (~135us):
            # the collective's completion fluctuates between ~95 and ~128us
            # run-to-run, and a stalled tail would drop the PE p-state
            sts = [st_load(g) for g in (6, 0, 1)]
            base_group(6, fused_st=sts[0])
            tail(0, sts[1])
            tail(1, sts[2])
            sts = [st_load(g) for g in (7, 2, 3)]
            base_group(7, fused_st=sts[0])
            tail(2, sts[1])
            tail(3, sts[2])
            sts = [st_load(g) for g in (4, 5)]
            tail(4, sts[0])
            tail(5, sts[1])
    return nc


_NC_CACHE = None


def build_in_maps(x, weight, bias, lora_a, lora_b, indices):
    bf = ml_dtypes.bfloat16
    f8 = mybir.dt.np(F8)

    # [128 h-partitions, group, kb, token] PE-tile-major layout
    xG = np.ascontiguousarray(
        x.astype(bf).reshape(G, TG, KB, 128).transpose(3, 0, 2, 1))
    aTr = np.ascontiguousarray(
        (lora_a * A_SCALE).astype(f8).reshape(LR, H).T
        .reshape(KB, 128, LR).transpose(1, 0, 2))                   # (128,KB,LR)
    idx_f = indices.astype(np.float32)                              # (T,)
    lrow = np.broadcast_to(
        (np.arange(128)[:, None] // 16).astype(np.float32), (128, 4)
    ).copy()
    lrow = lrow + (np.arange(4)[None, :] * 8).astype(np.float32)    # (128, 4)

    in_maps = []
    for c in range(N_CORES):
        wTc = np.ascontiguousarray(
            weight[c * O_SH:(c + 1) * O_SH, :].astype(bf).T
            .reshape(KB, 128, O_SH).transpose(1, 0, 2))             # (128,KB,O_SH)
        bTc = np.ascontiguousarray(
            (lora_b[:, c * O_SH:(c + 1) * O_SH, :] * B_SCALE).astype(f8)
            .transpose(0, 2, 1).reshape(LR, O_SH)                   # ((l,r), o)
            .reshape(4, 128, O_SH).transpose(1, 0, 2))              # (128,4,O_SH)
        bias_c = np.ascontiguousarray(
            bias[c * O_SH:(c + 1) * O_SH].astype(bf))[None, :]
        idx_bc = np.broadcast_to(
            idx_f[c * T_LOC:(c + 1) * T_LOC][None, :], (128, T_LOC)
        ).copy()
        xl_c = np.ascontiguousarray(
            x[c * T_LOC:(c + 1) * T_LOC, :].astype(f8).T
            .reshape(KB, 128, T_LOC).transpose(1, 0, 2))            # (128,KB,T_LOC)
        in_maps.append({
            "xG": xG, "xl_r": xl_c, "wTr": wTc, "aTr": aTr, "bTr": bTc,
            "bias_row": bias_c, "idx_bc": idx_bc, "lrow": lrow,
        })
    return in_maps


def kernel(x, weight, bias, lora_a, lora_b, indices):
    global _NC_CACHE
    in_maps = build_in_maps(x, weight, bias, lora_a, lora_b, indices)
    if _NC_CACHE is None:
        _NC_CACHE = _build()
    r = run_bass_kernel_spmd(_NC_CACHE, in_maps, core_ids=list(range(N_CORES)))
    return np.concatenate(
        [r.results[c]["out"].astype(np.float32) for c in range(N_CORES)], axis=1)


# revision 34
# speedup vs baseline: 1.0471x; 1.0312x over previous
"""ColumnParallelLinear + per-token LoRA (punica add_lora) on 8 NeuronCores.

out = x @ W^T + b + B[idx] @ (A[idx] @ x^T), idx==-1 skips LoRA.

Sharding: tensor-parallel over the output dim (vLLM ColumnParallelLinear):
weight, bias and lora_b are sharded 512-wide per core; lora_a and indices
are replicated. The per-token LoRA shrink (s = A @ x) is sharded over
tokens (256/core) and shared via an on-chip fp8 AllGather; the LoRA expand
is a dense matmul against the routing-masked shrink
(s_masked[t, (l,r)] = (idx[t]==l) * s[t, (l,r)]).

Perf notes (measured on HW, ~180us vs the 204us predecessor):
- The base matmul stays bf16 (pure fp8 fails the 2e-2 gate: measured
  3.9e-2). The shrink/expand run as fp8e4 DoubleRow matmuls (2 K-tiles per
  instruction, 2 elem/cycle), halving their PE occupancy; their
  quantization error only feeds the small LoRA correction (total ~6e-3).
- sm rides at scale s/4 and lora_b at x4 so the expand product lands at
  scale 1: the last two token groups accumulate their expand directly into
  the base psum group, and the separate expand tails add to the staged
  base+bias rows with a single scalar_tensor_tensor during the bf16 store.
- The first ~25us are chip-HBM-bound (all 8 cores streaming), so the PE
  start is floored; the streams ride three DMA queues (x groups on SP,
  weights on GpSimd, shrink/mask inputs on Activation) ordered so that
  later-needed bytes never delay earlier-needed ones.
- The collective has a ~60us bootstrap and its completion fluctuates
  ~95-128us run-to-run, so ALL AllGather-dependent work sits after base
  sweep 6 (~135us); st/ot buffers are deep enough that the four trailing
  tails pipeline without stalling the PE (whose p-state drops on any gap).
"""
import json

import numpy as np
import ml_dtypes

import concourse.bass as bass
import concourse.mybir as mybir
import concourse.tile as tile
from concourse.bass_utils import run_bass_kernel_spmd

T, H, O, L, R = 2048, 4096, 4096, 32, 16
N_CORES = 8
O_SH = O // N_CORES          # 512  output cols per core
T_LOC = T // N_CORES         # 256  tokens whose LoRA-shrink this core computes
KB = H // 128                # 32   contraction blocks
LR = L * R                   # 512  stacked (lora, rank) rows
G = 8                        # base-matmul token groups
TG = T // G                  # 256  tokens per group
N_FUSED = 2                  # trailing groups whose expand accumulates in-psum
BF16 = mybir.dt.bfloat16
F32 = mybir.dt.float32
F8 = mybir.dt.float8e4
DR = mybir.MatmulPerfMode.DoubleRow
A_SCALE = 16.0               # lora_a is pre-scaled x16 into fp8
B_SCALE = 4.0                # lora_b is pre-scaled x4 into fp8
SM_SCALE = 1.0 / (4.0 * A_SCALE)   # psum_s (16*s) -> sm (s/4); s/4 * 4b = s*b


def _split_waits(raw: bytes) -> bytes:
    """This walrus build rejects instructions carrying more than one sync
    wait ("Too many sync wait commands"), but Tile attaches one wait per
    producing proc. Hoist all but one wait of each instruction onto
    single-wait NoOps inserted just before it on the same engine — the
    engine executes its stream in order, so the gating is identical."""
    m = json.loads(raw)
    ctr = 0
    for f in m["functions"]:
        for b in f["blocks"]:
            out = []
            for inst in b["instructions"]:
                si = inst.get("sync_info")
                waits = si.get("on_wait") if si else None
                if waits and len(waits) > 1:
                    for w in waits[:-1]:
                        ctr += 1
                        out.append({
                            "debug": inst.get("debug", 0),
                            "engine": inst["engine"],
                            "ins": [],
                            "name": f"I-wsplit-{ctr}",
                            "opcode": "NoOp",
                            "outs": [],
                            "sync_info": {"on_update": [], "on_wait": [w]},
                        })
                    si["on_wait"] = [waits[-1]]
                out.append(inst)
            b["instructions"] = out
    return json.dumps(m).encode()


class _WaitSplitBass(bass.Bass):
    def to_json_bytes(self) -> bytes:
        return _split_waits(super().to_json_bytes())


def _build() -> bass.Bass:
    nc = _WaitSplitBass()
    # all streamed inputs are PE-tile-major: [128 h-partitions, ..., free]
    xG = nc.dram_tensor("xG", [128, G, KB, TG], BF16, kind="ExternalInput")
    xl_r = nc.dram_tensor("xl_r", [128, KB, T_LOC], F8, kind="ExternalInput")
    wTr = nc.dram_tensor("wTr", [128, KB, O_SH], BF16, kind="ExternalInput")
    aTr = nc.dram_tensor("aTr", [128, KB, LR], F8, kind="ExternalInput")
    bTr = nc.dram_tensor("bTr", [128, 4, O_SH], F8, kind="ExternalInput")
    bias_row = nc.dram_tensor("bias_row", [1, O_SH], BF16, kind="ExternalInput")
    idx_bc_d = nc.dram_tensor("idx_bc", [128, T_LOC], F32, kind="ExternalInput")
    lrow_d = nc.dram_tensor("lrow", [128, 4], F32, kind="ExternalInput")
    out = nc.dram_tensor("out", [T, O_SH], BF16, kind="ExternalOutput")

    with tile.TileContext(nc) as tc:
        with (
            tc.tile_pool(name="res", bufs=1) as res,          # long-lived SBUF
            tc.tile_pool(name="stream", bufs=4) as stream,    # streamed SBUF
            tc.tile_pool(name="ps", bufs=2, space="PSUM") as ps,
            tc.tile_pool(name="dram", bufs=1, space="DRAM") as dram,
        ):
            # ------------- tiny mask inputs first, then the shrink inputs,
            # all on the Activation queue.  The AllGather completion is
            # bootstrap-floored at ~95-110us, so the shrink stream needs no
            # early bandwidth: the base-matmul x/w streams get priority ------
            bias_r = res.tile([1, O_SH], BF16, name="bias_r")
            nc.scalar.dma_start(bias_r[:], bias_row[:])
            xl_all = res.tile([128, KB, T_LOC], F8, name="xl_all")
            at_all = res.tile([128, KB, LR], F8, name="at_all")
            for lo, hi in ((0, 8), (8, 16), (16, 24), (24, 32)):
                nc.scalar.dma_start(xl_all[:, lo:hi, :], xl_r[:, lo:hi, :])
            for lo, hi in ((0, 8), (8, 16), (16, 24), (24, 32)):
                nc.scalar.dma_start(at_all[:, lo:hi, :], aTr[:, lo:hi, :])
            idx_bc = res.tile([128, T_LOC], F32, name="idx_bc_t")
            nc.scalar.dma_start(idx_bc[:], idx_bc_d[:])
            lrow = res.tile([128, 4], F32, name="lrow_t")
            nc.scalar.dma_start(lrow[:], lrow_d[:])

            # ------------- weights + lora_b on the GpSimd queue -------------
            wt_all = res.tile([128, KB, O_SH], BF16, name="wt_all")
            for c in range(4):
                nc.gpsimd.dma_start(wt_all[:, 8 * c:8 * (c + 1), :],
                                    wTr[:, 8 * c:8 * (c + 1), :])
            bt_all = res.tile([128, 4, O_SH], F8, name="bt_all")
            nc.gpsimd.dma_start(bt_all[:], bTr[:])

            # ------------- x groups on the SP queue -------------------------
            xs0 = stream.tile([128, KB, TG], BF16, name="xs", tag="xs", bufs=3)
            nc.sync.dma_start(xs0[:, 0:16, :], xG[:, 0:1, 0:16, :])
            nc.sync.dma_start(xs0[:, 16:32, :], xG[:, 0:1, 16:32, :])

            ones_t = res.tile([1, O_SH], BF16, name="ones_t")
            nc.vector.memset(ones_t[:], 1.0)

            # base accumulations of non-fused groups land in SBUF (with bias)
            # as each group finishes; their expand is combined during store.
            base_sb = res.tile([128, 2 * (G - N_FUSED) * O_SH], F32,
                               name="base_sb")

            ps_s = [ps.tile([128, T_LOC], F32, name=f"ps_s{m}", tag=tg)
                    for m, tg in enumerate(["psd0", "psd1", "psd0", "psd1"])]

            # Preamble: bias broadcast (K=1 ones-matmul, also warms the PE),
            # then base groups 0-1 on the early bandwidth; the fp8 DoubleRow
            # shrink (each matmul contracts the kb pair (2k2, 2k2+1)) runs
            # after group 1, warm and with its trickled inputs surely landed.
            bias_ps = ps.tile([128, O_SH], F32, name="bias_ps", tag="pso0")
            nc.tensor.matmul(bias_ps[:], ones_t[:, 0:128], bias_r[:],
                             start=True, stop=True)
            bias_bc = res.tile([128, O_SH], F32, name="bias_bc")
            nc.vector.tensor_copy(bias_bc[:], bias_ps[:])

            ps_o0 = [ps.tile([128, O_SH], F32, name=f"ps_o0_{t}", tag=f"pso{t}")
                     for t in range(2)]

            def base0_chunk(lo, hi):
                for kb in range(lo, hi):
                    for tt in range(2):
                        nc.tensor.matmul(
                            ps_o0[tt][:],
                            xs0[:, kb, tt * 128:(tt + 1) * 128],
                            wt_all[:, kb, :],
                            start=(kb == 0),
                            stop=(kb == KB - 1),
                        )

            def shrink_chunk(c):
                for k2 in range(4 * c, 4 * (c + 1)):
                    for m in range(4):
                        nc.tensor.matmul(
                            ps_s[m][:],
                            at_all[:, 2 * k2:2 * k2 + 2, m * 128:(m + 1) * 128],
                            xl_all[:, 2 * k2:2 * k2 + 2, :],
                            start=(k2 == 0),
                            stop=(k2 == 15),
                            perf_mode=DR,
                        )

            base0_chunk(0, 32)
            for tt in range(2):
                nc.vector.tensor_tensor(
                    base_sb[:, tt * O_SH:(tt + 1) * O_SH],
                    ps_o0[tt][:],
                    bias_bc[:],
                    op=mybir.AluOpType.add,
                )

            def mask_and_allgather():
                # routing mask, rescale + fp8 downcast: sm = (idx==l(p)) * s/4
                sm_f32 = res.tile([128, 4 * T_LOC], F32, name="sm_f32")
                for m in range(4):
                    nc.vector.scalar_tensor_tensor(
                        sm_f32[:, m * T_LOC:(m + 1) * T_LOC],
                        idx_bc[:],
                        lrow[:, m:m + 1],
                        ps_s[m][:],
                        op0=mybir.AluOpType.is_equal,
                        op1=mybir.AluOpType.mult,
                    )
                sm = res.tile([128, 4 * T_LOC], F8, name="sm")
                nc.vector.tensor_scalar_mul(sm[:], sm_f32[:], SM_SCALE)
                cc_in = dram.tile([LR, T_LOC], F8, name="cc_in")
                nc.gpsimd.dma_start(
                    cc_in[:].rearrange("(m p) t -> p m t", p=128),
                    sm[:].rearrange("p (m t) -> p m t", t=T_LOC),
                )
                cc_out = dram.tile([N_CORES, LR, T_LOC], F8, name="cc_out",
                                   addr_space="Shared")
                nc.gpsimd.collective_compute(
                    "AllGather",
                    mybir.AluOpType.bypass,
                    replica_groups=[list(range(N_CORES))],
                    ins=[cc_in.opt()],
                    outs=[cc_out.opt()],
                )
                return cc_out

            cc_out = [None]

            def st_load(g):
                st = stream.tile([128, 4, TG], F8, name="st", tag="st", bufs=4)
                for blk in range(4):
                    # scalar queue: idle by now, and unlike the gpsimd queue
                    # it is not blocked behind the collective
                    nc.scalar.dma_start(
                        st[:, blk:blk + 1, :],
                        cc_out[0][g, blk * 128:(blk + 1) * 128, :],
                    )
                return st

            def expand_mms(ps_into, st, start, stop):
                for tt in range(2):
                    for d2 in range(2):
                        nc.tensor.matmul(
                            ps_into[tt][:],
                            st[:, 2 * d2:2 * d2 + 2, tt * 128:(tt + 1) * 128],
                            bt_all[:, 2 * d2:2 * d2 + 2, :],
                            start=start and (d2 == 0),
                            stop=stop and (d2 == 1),
                            perf_mode=DR,
                        )

            def base_group(g, fused_st=None):
                ps_o = [ps.tile([128, O_SH], F32, name=f"ps_o{g}_{t}",
                                tag=f"pso{t}") for t in range(2)]
                xs = stream.tile([128, KB, TG], BF16, name="xs", tag="xs", bufs=3)
                if g <= 2:
                    # halves: the first 16 kb land ~5us earlier in the
                    # bandwidth-crunched early window
                    nc.sync.dma_start(xs[:, 0:16, :], xG[:, g:g + 1, 0:16, :])
                    nc.sync.dma_start(xs[:, 16:32, :], xG[:, g:g + 1, 16:32, :])
                else:
                    nc.sync.dma_start(xs[:], xG[:, g:g + 1, :, :])
                for kb in range(KB):
                    for tt in range(2):
                        nc.tensor.matmul(
                            ps_o[tt][:],
                            xs[:, kb, tt * 128:(tt + 1) * 128],
                            wt_all[:, kb, :],
                            start=(kb == 0),
                            stop=(fused_st is None) and (kb == KB - 1),
                        )
                if fused_st is not None:
                    # expand accumulates straight into the base psum (sm and
                    # bt scales multiply to exactly 1), then bias + store
                    expand_mms(ps_o, fused_st, start=False, stop=True)
                    ot = stream.tile([128, 2 * O_SH], BF16, name="ot",
                                     tag="ot", bufs=4)
                    for tt in range(2):
                        nc.vector.tensor_tensor(
                            ot[:, tt * O_SH:(tt + 1) * O_SH],
                            ps_o[tt][:],
                            bias_bc[:],
                            op=mybir.AluOpType.add,
                        )
                    for tt in range(2):
                        nc.sync.dma_start(
                            out[g * TG + tt * 128:g * TG + (tt + 1) * 128, :],
                            ot[:, tt * O_SH:(tt + 1) * O_SH])
                else:
                    for tt in range(2):
                        nc.vector.tensor_tensor(
                            base_sb[:, (2 * g + tt) * O_SH:
                                    (2 * g + tt + 1) * O_SH],
                            ps_o[tt][:],
                            bias_bc[:],
                            op=mybir.AluOpType.add,
                        )

            def tail(g, st):
                ps_d = [
                    ps.tile([128, O_SH], F32, name=f"ps_d{g}_{t}", tag=f"psd{t}")
                    for t in range(2)
                ]
                expand_mms(ps_d, st, start=True, stop=True)
                ot = stream.tile([128, 2 * O_SH], BF16, name="ot", tag="ot", bufs=4)
                for tt in range(2):
                    nc.vector.scalar_tensor_tensor(
                        ot[:, tt * O_SH:(tt + 1) * O_SH],
                        ps_d[tt][:],
                        1.0,
                        base_sb[:, (2 * g + tt) * O_SH:(2 * g + tt + 1) * O_SH],
                        op0=mybir.AluOpType.mult,
                        op1=mybir.AluOpType.add,
                    )
                dst = out[g * TG:(g + 1) * TG, :].rearrange(
                    "(tt p) o -> p tt o", p=128
                )
                nc.sync.dma_start(dst, ot[:].rearrange("p (tt o) -> p tt o", o=O_SH))

            # flow: groups 0-5 plain, 6-7 fused.  Shrink chunks bridge the
            # windows where the next x chunk is still in flight; separate
            # expand tails slot in from group 5 on, by when the AllGather
            # has always landed.
            base_group(1)
            for c in range(4):
                shrink_chunk(c)
            cc_out[0] = mask_and_allgather()
            base_group(2)
            base_group(3)
            base_group(4)
            base_group(5)
            # all AllGather-dependent work sits after base sweep 6 (~135us):
            # the collective's completion fluctuates between ~95 and ~128us
            # run-to-run, and a stalled tail would drop the PE p-state
            sts = [st_load(g) for g in (6, 0, 1)]
            base_group(6, fused_st=sts[0])
            tail(0, sts[1])
            tail(1, sts[2])
            sts = [st_load(g) for g in (7, 2, 3)]
            base_group(7, fused_st=sts[0])
            tail(2, sts[1])
            tail(3, sts[2])
            sts = [st_load(g) for g in (4, 5)]
            tail(4, sts[0])
            tail(5, sts[1])
    return nc


_NC_CACHE = None


def build_in_maps(x, weight, bias, lora_a, lora_b, indices):
    bf = ml_dtypes.bfloat16
    f8 = mybir.dt.np(F8)

    # [128 h-partitions, group, kb, token] PE-tile-major layout
    xG = np.ascontiguousarray(
        x.astype(bf).reshape(G, TG, KB, 128).transpose(3, 0, 2, 1))
    aTr = np.ascontiguousarray(
        (lora_a * A_SCALE).astype(f8).reshape(LR, H).T
        .reshape(KB, 128, LR).transpose(1, 0, 2))                   # (128,KB,LR)
    idx_f = indices.astype(np.float32)                              # (T,)
    lrow = np.broadcast_to(
        (np.arange(128)[:, None] // 16).astype(np.float32), (128, 4)
    ).copy()
    lrow = lrow + (np.arange(4)[None, :] * 8).astype(np.float32)    # (128, 4)

    in_maps = []
    for c in range(N_CORES):
        wTc = np.ascontiguousarray(
            weight[c * O_SH:(c + 1) * O_SH, :].astype(bf).T
            .reshape(KB, 128, O_SH).transpose(1, 0, 2))             # (128,KB,O_SH)
        bTc = np.ascontiguousarray(
            (lora_b[:, c * O_SH:(c + 1) * O_SH, :] * B_SCALE).astype(f8)
            .transpose(0, 2, 1).reshape(LR, O_SH)                   # ((l,r), o)
            .reshape(4, 128, O_SH).transpose(1, 0, 2))              # (128,4,O_SH)
        bias_c = np.ascontiguousarray(
            bias[c * O_SH:(c + 1) * O_SH].astype(bf))[None, :]
        idx_bc = np.broadcast_to(
            idx_f[c * T_LOC:(c + 1) * T_LOC][None, :], (128, T_LOC)
        ).copy()
        xl_c = np.ascontiguousarray(
            x[c * T_LOC:(c + 1) * T_LOC, :].astype(f8).T
            .reshape(KB, 128, T_LOC).transpose(1, 0, 2))            # (128,KB,T_LOC)
        in_maps.append({
            "xG": xG, "xl_r": xl_c, "wTr": wTc, "aTr": aTr, "bTr": bTc,
            "bias_row": bias_c, "idx_bc": idx_bc, "lrow": lrow,
        })
    return in_maps


def kernel(x, weight, bias, lora_a, lora_b, indices):
    global _NC_CACHE
    in_maps = build_in_maps(x, weight, bias, lora_a, lora_b, indices)
    if _NC_CACHE is None:
        _NC_CACHE = _build()
    r = run_bass_kernel_spmd(_NC_CACHE, in_maps, core_ids=list(range(N_CORES)))
    return np.concatenate(
        [r.results[c]["out"].astype(np.float32) for c in range(N_CORES)], axis=1)
